# revision 3
# baseline (speedup 1.0000x reference)
"""Trainium2 Bass kernel for nn_KnowledgeFusion.

Math (b=8, H=W=32, d=o=256, n_obj=15, n=16 with appended mean-emb):
  embs_aug = concat([embs, mean(embs)])                  [b,16,256]
  mask     = rasterized boxes (rounded to PATCH_SIZE=2)  [b,16,1024] in {0,1}
  proj     = patches @ Wp                                [b,1024,256]
  inj      = embs_aug @ We                               [b,16,256]
  s[hw]    = sum_n mask[n,hw]   (>=1: image box row)
  out      = proj + (mask^T @ inj) / s[:,None]           [b,1024,256]

(The reference's (proj + m*inj) masked-mean collapses to this because
mask^2 == mask.)

Sharding: data-parallel over batch; core c computes batch c. Wp/We
replicated. patches/embs are transposed host-side so the contraction dim
(d) lands on SBUF partitions for the PE matmuls.
"""

import sys

sys.path.insert(0, "/opt/trn_rl_repo")

import numpy as np

import concourse.bass as bass
import concourse.bacc as bacc
import concourse.mybir as mybir
from concourse import tile
from concourse import bass_utils
from concourse.alu_op_type import AluOpType

B, H, W, D = 8, 32, 32, 256
NOBJ, N = 15, 16
HW = H * W
O = 256
FP = mybir.dt.float32
I32 = mybir.dt.int32
AF = mybir.ActivationFunctionType
AX = mybir.AxisListType


def build_nc(debug: bool = False):
    nc = bacc.Bacc("TRN2", target_bir_lowering=False, debug=debug, num_devices=B)

    pT = nc.dram_tensor("pT", [D, HW], FP, kind="ExternalInput")
    eT = nc.dram_tensor("eT", [D, NOBJ], FP, kind="ExternalInput")
    loc = nc.dram_tensor("loc", [N, 4], I32, kind="ExternalInput")
    Wp = nc.dram_tensor("Wp", [D, O], FP, kind="ExternalInput")
    We = nc.dram_tensor("We", [D, O], FP, kind="ExternalInput")
    out = nc.dram_tensor("out", [HW, O], FP, kind="ExternalOutput")

    with tile.TileContext(nc) as tc:
        with (
            tc.tile_pool(name="big", bufs=1) as big,
            tc.tile_pool(name="small", bufs=1) as small,
            tc.tile_pool(name="outp", bufs=3) as outp,
            tc.tile_pool(name="psP", bufs=2, space=bass.MemorySpace.PSUM) as psP,
            tc.tile_pool(name="psA", bufs=2, space=bass.MemorySpace.PSUM) as psA,
            tc.tile_pool(name="psS", bufs=2, space=bass.MemorySpace.PSUM) as psS,
        ):
            # ---- weight / activation loads (d on partitions, 2 chunks of 128)
            pT_sb = [big.tile([128, HW], FP, tag=f"pt{k}", name=f"pt{k}") for k in range(2)]
            for k in range(2):
                for h in range(2):
                    nc.sync.dma_start(
                        pT_sb[k][:, 512 * h : 512 * (h + 1)],
                        pT[128 * k : 128 * (k + 1), 512 * h : 512 * (h + 1)],
                    )
            Wp_sb = [small.tile([128, O], FP, tag=f"wp{k}", name=f"wp{k}") for k in range(2)]
            We_sb = [small.tile([128, O], FP, tag=f"we{k}", name=f"we{k}") for k in range(2)]
            eT_sb = [small.tile([128, N], FP, tag=f"et{k}", name=f"et{k}") for k in range(2)]
            for k in range(2):
                nc.sync.dma_start(Wp_sb[k][:], Wp[128 * k : 128 * (k + 1), :])
                nc.sync.dma_start(We_sb[k][:], We[128 * k : 128 * (k + 1), :])
                nc.sync.dma_start(eT_sb[k][:, 0:NOBJ], eT[128 * k : 128 * (k + 1), :])
                # col 15 = mean of the 15 object embeddings
                nc.vector.tensor_reduce(
                    eT_sb[k][:, NOBJ : NOBJ + 1], eT_sb[k][:, 0:NOBJ], AX.X, AluOpType.add
                )
                nc.vector.tensor_scalar_mul(
                    eT_sb[k][:, NOBJ : NOBJ + 1], eT_sb[k][:, NOBJ : NOBJ + 1], 1.0 / NOBJ
                )

            # ---- inj = embs_aug @ We  -> [16, 256]
            psumI = psA.tile([N, O], FP, tag="psI")
            nc.tensor.matmul(psumI[:], eT_sb[0][:], We_sb[0][:], start=True, stop=False)
            nc.tensor.matmul(psumI[:], eT_sb[1][:], We_sb[1][:], start=False, stop=True)
            inj_sb = small.tile([N, O], FP)
            nc.scalar.activation(inj_sb[:], psumI[:], AF.Copy)

            # ---- boxes: round starts down / ends up to multiples of 2
            loc_sb = small.tile([N, 4], I32)
            nc.sync.dma_start(loc_sb[:], loc[:])
            locm = small.tile([N, 4], I32)
            nc.vector.tensor_scalar(locm[:], loc_sb[:], 1, None, op0=AluOpType.bitwise_and)
            boxes_i = small.tile([N, 4], I32)
            nc.vector.tensor_tensor(boxes_i[:], loc_sb[:], locm[:], op=AluOpType.subtract)
            nc.vector.tensor_scalar_add(boxes_i[:, 2:4], boxes_i[:, 2:4], 2)
            boxes_f = small.tile([N, 4], FP)
            nc.vector.tensor_copy(boxes_f[:], boxes_i[:])

            # ---- row/col interval masks [16, 32]
            grid_i = small.tile([N, 32], I32)
            nc.gpsimd.iota(grid_i[:], pattern=[[1, 32]], base=0, channel_multiplier=0)
            grid_f = small.tile([N, 32], FP)
            nc.vector.tensor_copy(grid_f[:], grid_i[:])

            rowm = small.tile([N, 32], FP)
            colm = small.tile([N, 32], FP)
            tmp = small.tile([N, 32], FP, tag="cmp_tmp")
            nc.vector.tensor_scalar(tmp[:], grid_f[:], boxes_f[:, 2:3], None, op0=AluOpType.is_lt)
            nc.vector.scalar_tensor_tensor(
                rowm[:], grid_f[:], boxes_f[:, 0:1], tmp[:], op0=AluOpType.is_ge, op1=AluOpType.mult
            )
            tmp2 = small.tile([N, 32], FP, tag="cmp_tmp2")
            nc.vector.tensor_scalar(tmp2[:], grid_f[:], boxes_f[:, 3:4], None, op0=AluOpType.is_lt)
            nc.vector.scalar_tensor_tensor(
                colm[:], grid_f[:], boxes_f[:, 1:2], tmp2[:], op0=AluOpType.is_ge, op1=AluOpType.mult
            )

            # ---- mask [16, 1024]: mask[n, y*32+x] = rowm[n,y] * colm[n,x]
            mask_sb = small.tile([N, HW], FP)
            for y in range(H):
                nc.vector.tensor_scalar_mul(
                    mask_sb[:, W * y : W * (y + 1)], colm[:], rowm[:, y : y + 1]
                )

            ones_sb = small.tile([N, 1], FP)
            nc.vector.memset(ones_sb[:], 1.0)

            # ---- main loop: 8 tiles of 128 pixels
            for t in range(8):
                s0 = 128 * t
                psumP = psP.tile([128, O], FP, tag="psP")
                nc.tensor.matmul(
                    psumP[:], pT_sb[0][:, s0 : s0 + 128], Wp_sb[0][:], start=True, stop=False
                )
                nc.tensor.matmul(
                    psumP[:], pT_sb[1][:, s0 : s0 + 128], Wp_sb[1][:], start=False, stop=True
                )
                psumA = psA.tile([128, O], FP, tag="psA")
                nc.tensor.matmul(
                    psumA[:], mask_sb[:, s0 : s0 + 128], inj_sb[:], start=True, stop=True
                )
                psumS = psS.tile([128, 1], FP, tag="psS")
                nc.tensor.matmul(
                    psumS[:], mask_sb[:, s0 : s0 + 128], ones_sb[:], start=True, stop=True
                )
                rec = outp.tile([128, 1], FP, tag="rec")
                nc.vector.reciprocal(rec[:], psumS[:])
                proj_sb = outp.tile([128, O], FP, tag="proj")
                nc.scalar.activation(proj_sb[:], psumP[:], AF.Copy)
                out_sb = outp.tile([128, O], FP, tag="out")
                nc.vector.scalar_tensor_tensor(
                    out_sb[:], psumA[:], rec[:, 0:1], proj_sb[:],
                    op0=AluOpType.mult, op1=AluOpType.add,
                )
                nc.sync.dma_start(out[s0 : s0 + 128, :], out_sb[:])

    nc.compile()
    return nc


def make_in_maps(inputs):
    patches = np.asarray(inputs["patches"], dtype=np.float32)
    embs = np.asarray(inputs["embs"], dtype=np.float32)
    locations = np.asarray(inputs["locations"], dtype=np.int32)
    Wp = np.ascontiguousarray(np.asarray(inputs["Wp"], dtype=np.float32))
    We = np.ascontiguousarray(np.asarray(inputs["We"], dtype=np.float32))
    img_box = np.array([[0, 0, H, W]], dtype=np.int32)
    in_maps = []
    for b in range(B):
        in_maps.append(
            {
                "pT": np.ascontiguousarray(patches[b].reshape(HW, D).T),
                "eT": np.ascontiguousarray(embs[b].T),
                "loc": np.ascontiguousarray(np.concatenate([locations[b], img_box], 0)),
                "Wp": Wp,
                "We": We,
            }
        )
    return in_maps


_NC = None


def _get_nc():
    global _NC
    if _NC is None:
        _NC = build_nc(debug=False)
    return _NC


def run(inputs, trace: bool = False, **kwargs):
    nc = _get_nc()
    res = bass_utils.run_bass_kernel_spmd(
        nc, make_in_maps(inputs), core_ids=list(range(B)), trace=trace, **kwargs
    )
    full = np.stack([res.results[b]["out"] for b in range(B)], axis=0)
    return full.astype(np.float32), res


def kernel(**inputs) -> np.ndarray:
    full, _ = run(inputs, trace=False)
    return full
